# revision 3
# baseline (speedup 1.0000x reference)
"""Trainium2 Bass kernel for a Tsit5 NeuralODE (MLP vector field), v2.

Contract: kernel(**inputs) takes the FULL inputs of reference.setup_inputs()
and returns the FULL [101, 4096, 64] trajectory. Data-parallel: batch 4096 ->
8 x 512 cores; each core integrates 100 Tsit5 steps (6 MLP evals each),
unrolled into one NEFF.

Structure (vs v1 baseline):
  - 2 independent batch chains per core (2 x 256 columns), emitted with a
    3-stage skew, so TensorE / ScalarE / VectorE have a second independent
    instruction stream to overlap with instead of ping-ponging on one
    dependency chain.
  - z-augmentation: stage-input tiles are [128, CN] f32r whose row 64 is
    constant 1.0 (rows 65..127 zero); W0 is augmented with b0 as row 64 and
    zero-padded to K=128 (HW-measured: K=64 matmuls cost ~475ns vs ~321ns
    at K=128). Layer-0 tanh then needs no act bias and runs as ONE merged
    activation over both m-tiles from a single PSUM bank per chain.
  - All RK fold/setup ops stay on VectorE (GpSimd tensor_scalar measured
    ~7.5us/op on HW - 15x slower than DVE - so the Pool offload idea is a
    trap). y state is carried in fp32 (separate yn tile) for precision; the
    f32r zaug twin feeds the matmuls.
  - PSUM (8 banks): per chain ps0, ps1, ps2 [128,2,CN] one bank each +
    ps3 [64,2,CN] one bank.
  - The Tile list-scheduler runs with HW-calibrated cost constants
    (PE cycle incl. LDWEIGHTS, DVE access latency, GpSimd efficiency)
    patched in during build only, so its greedy order reflects reality.
Host side (numpy): shard + transpose y0, build the h_t*A_ij / h_t*c_i*b3
tables from ts, augment W0 with b0, transpose/gather the output.
"""

import numpy as np

import concourse.bass as bass
import concourse.tile as tile
from concourse import bacc, mybir
from concourse.bass_utils import run_bass_kernel_spmd

# Tsit5 tableau (must match the reference)
A21 = 0.161
A31, A32 = -0.008480655492356989, 0.335480655492357
A41, A42, A43 = 2.8971530571054935, -6.359448489975075, 4.3622954328695815
A51, A52, A53, A54 = 5.325864828439257, -11.748883564062828, 7.4955393428898365, -0.09249506636175525
A61, A62, A63, A64, A65 = 5.86145544294642, -12.92096931784711, 8.159367898576159, -0.071584973281401, -0.028269050394068383
B1, B2, B3, B4, B5, B6 = 0.09646076681806523, 0.01, 0.4798896504144996, 1.379008574103742, -3.290069515436081, 2.324710524099774

ACOEF = {
    2: {1: A21},
    3: {1: A31, 2: A32},
    4: {1: A41, 2: A42, 3: A43},
    5: {1: A51, 2: A52, 3: A53, 4: A54},
    6: {1: A61, 2: A62, 3: A63, 4: A64, 5: A65},
}
BCOEF = {1: B1, 2: B2, 3: B3, 4: B4, 5: B5, 6: B6}

# pre-acc pairs (i, j) with j <= i-2, flat order for the hA table
PAIRS = [(i, j) for j in range(1, 5) for i in range(j + 2, 7)]
PAIRQ = {p: q for q, p in enumerate(PAIRS)}

NCORES = 8
DIM, WIDTH = 64, 256
BATCH, NT = 4096, 101
NSTEP = NT - 1
SHARD = BATCH // NCORES      # 512 rows per core

F32 = mybir.dt.float32
F32R = mybir.dt.float32r
MULT = mybir.AluOpType.mult
ADD = mybir.AluOpType.add
TANH = mybir.ActivationFunctionType.Tanh

_cache = {}


_SPEC_PATCH = {
    "PE_CYCLE": 1e9 / 1.6e9,          # measured mm512 f32r ~321ns incl LDW
    "CYCLE_T": None,                   # filled below
    "ACCESS_CYCLES": None,
    "GPSIMD_IMPL_EFFICIENCY_DEFAULT": 0.06,   # measured tensor_scalar ~7.5us
}


def _patched_spec():
    from concourse.hw_specs import TRN2Spec
    import concourse.mybir as _mb
    import concourse.bass as _bass
    old = {}
    patch = dict(_SPEC_PATCH)
    patch["CYCLE_T"] = {**TRN2Spec.CYCLE_T, _mb.EngineType.DVE: 1e9 / 0.96e9}
    patch["ACCESS_CYCLES"] = {
        **TRN2Spec.ACCESS_CYCLES,
        (_bass.MemorySpace.SBUF, _mb.EngineType.DVE): 160,
        (_bass.MemorySpace.PSUM, _mb.EngineType.DVE): 160,
    }
    for k, v in patch.items():
        old[k] = getattr(TRN2Spec, k)
        setattr(TRN2Spec, k, v)
    return TRN2Spec, old


def _build(nsteps=NSTEP, nch=2, pool_setup=False, skew=3, simple=True):
    spec, saved = _patched_spec()
    try:
        return _build_inner(nsteps, nch, pool_setup, skew, simple)
    finally:
        for k, v in saved.items():
            setattr(spec, k, v)


def _build_inner(nsteps, nch, pool_setup, skew, simple):
    CN = SHARD // nch
    nc = bacc.Bacc("TRN2", target_bir_lowering=False, debug=False, num_devices=NCORES)

    y0t_d = nc.dram_tensor("y0t", [DIM, SHARD], F32, kind="ExternalInput").ap()
    hA_d = nc.dram_tensor("hA", [DIM, 21 * nsteps], F32, kind="ExternalInput").ap()
    hc_d = nc.dram_tensor("hc", [DIM, 6 * nsteps], F32, kind="ExternalInput").ap()
    w0a_d = nc.dram_tensor("W0A", [128, WIDTH], F32, kind="ExternalInput").ap()
    w1_d = nc.dram_tensor("W1", [WIDTH, WIDTH], F32, kind="ExternalInput").ap()
    w2_d = nc.dram_tensor("W2", [WIDTH, WIDTH], F32, kind="ExternalInput").ap()
    w3_d = nc.dram_tensor("W3", [WIDTH, DIM], F32, kind="ExternalInput").ap()
    b1_d = nc.dram_tensor("b1", [WIDTH], F32, kind="ExternalInput").ap()
    b2_d = nc.dram_tensor("b2", [WIDTH], F32, kind="ExternalInput").ap()
    out_d = nc.dram_tensor("ysT", [nsteps, DIM, SHARD], F32, kind="ExternalOutput").ap()

    setup_eng = "gpsimd" if pool_setup else "vector"

    with tile.TileContext(nc) as tc:
        with tc.tile_pool(name="const", bufs=1) as const, \
             tc.tile_pool(name="state", bufs=2) as state, \
             tc.tile_pool(name="zpool", bufs=1) as zpool, \
             tc.tile_pool(name="accp", bufs=1) as accp, \
             tc.tile_pool(name="work", bufs=2) as work, \
             tc.tile_pool(name="psum", bufs=1, space="PSUM") as psum:

            # ---- load + round weights to f32r ----
            # W0 augmented with b0 as row 64: [65, 2, 128]
            w0s = const.tile([128, 2, 128], F32, tag="w0s")
            nc.sync.dma_start(w0s[:], w0a_d.rearrange("k (m j) -> k m j", j=128))
            w0 = const.tile([128, 2, 128], F32R, tag="w0")
            nc.vector.tensor_copy(w0[:], w0s[:])

            w1 = const.tile([128, 2, 2, 128], F32R, tag="w1")
            w2 = const.tile([128, 2, 2, 128], F32R, tag="w2")
            for wd, wt, nm in ((w1_d, w1, "w1"), (w2_d, w2, "w2")):
                ws = const.tile([128, 2, 2, 128], F32, tag=nm + "s", name=nm + "s")
                for t in range(2):
                    nc.sync.dma_start(
                        ws[:, t],
                        wd[t * 128:(t + 1) * 128, :].rearrange("k (m j) -> k m j", j=128),
                    )
                nc.vector.tensor_copy(wt[:], ws[:])

            w3s = const.tile([128, 2, DIM], F32, tag="w3s")
            nc.sync.dma_start(w3s[:], w3_d.rearrange("(t k) d -> k t d", k=128))
            w3 = const.tile([128, 2, DIM], F32R, tag="w3")
            nc.vector.tensor_copy(w3[:], w3s[:])

            # ---- biases as [128, 2] (column m = Mtile m) ----
            bt = {}
            for bd, nm in ((b1_d, "b1"), (b2_d, "b2")):
                tile_b = const.tile([128, 2], F32, tag=nm + "t", name=nm + "t")
                nc.sync.dma_start(tile_b[:], bd.rearrange("(m p) -> p m", p=128))
                bt[nm] = tile_b

            # ---- per-step scalar tables ----
            hA = const.tile([DIM, 21 * nsteps], F32, tag="hA")
            nc.sync.dma_start(hA[:], hA_d)
            hc = const.tile([DIM, 6 * nsteps], F32, tag="hc")
            nc.sync.dma_start(hc[:], hc_d)

            # ---- initial state (per chain) ----
            y0s = const.tile([DIM, SHARD], F32, tag="y0s")
            nc.sync.dma_start(y0s[:], y0t_d)

            zaug, acc, accy = [], [], []
            for c in range(nch):
                za = zpool.tile([128, CN], F32R, tag=f"z{c}", name=f"z{c}")
                nc.gpsimd.memset(za[DIM:128, :].bitcast(F32), 0.0)
                nc.gpsimd.memset(za[DIM:DIM + 1, :].bitcast(F32), 1.0)
                nc.vector.tensor_copy(za[0:DIM, :], y0s[:, c * CN:(c + 1) * CN])
                zaug.append(za)
                a = {}
                for i in range(2, 7):
                    a[i] = accp.tile([DIM, CN], F32, tag=f"acc{i}_{c}", name=f"acc{i}_{c}")
                acc.append(a)
                ay = accp.tile([DIM, CN], F32, tag=f"accy_{c}", name=f"accy_{c}")
                accy.append(ay)

            def sA(q, t):
                return hA[:, q * nsteps + t: q * nsteps + t + 1]

            def sC(q, t):
                return hc[:, q * nsteps + t: q * nsteps + t + 1]

            def emit_setup(c, ysrc, t):
                eng = getattr(nc, setup_eng)
                for i in range(2, 7):
                    eng.tensor_scalar(acc[c][i][:], ysrc, sC(i - 2, t), None, ADD)
                eng.tensor_scalar(accy[c][:], ysrc, sC(5, t), None, ADD)

            # prologue: accumulators for step 0 from y0
            for c in range(nch):
                emit_setup(c, y0s[:, c * CN:(c + 1) * CN], 0)

            def emit_stage(c, g):
                """Emit all engine work for chain c, global stage index g
                (g = 6*t + s-1)."""
                t, s1 = divmod(g, 6)
                s = s1 + 1
                # ---- layer 0 (K=65, bias folded into w0 row 64) ----
                ps0 = psum.tile([128, 2, CN], F32, tag=f"ps0_{c}", name=f"ps0_{c}")
                for m in range(2):
                    nc.tensor.matmul(ps0[:, m], w0[:, m], zaug[c][:],
                                     start=True, stop=True)
                h0 = work.tile([128, 2, CN], F32R, tag=f"h0_{c}", name=f"h0_{c}")
                nc.scalar.activation(h0[:], ps0[:], TANH)

                # ---- layer 1 ----
                ps1 = psum.tile([128, 2, CN], F32, tag=f"ps1_{c}", name=f"ps1_{c}")
                for m in range(2):
                    for k in range(2):
                        nc.tensor.matmul(ps1[:, m], w1[:, k, m], h0[:, k],
                                         start=(k == 0), stop=(k == 1))
                h1 = work.tile([128, 2, CN], F32R, tag=f"h1_{c}", name=f"h1_{c}")
                for m in range(2):
                    nc.scalar.activation(h1[:, m], ps1[:, m], TANH,
                                         bias=bt["b1"][:, m:m + 1])

                # ---- layer 2 ----
                ps2 = psum.tile([128, 2, CN], F32,
                                tag=(f"ps2_{c}" if simple else f"ps0_{c}"),
                                name=f"ps2_{c}")
                for m in range(2):
                    for k in range(2):
                        nc.tensor.matmul(ps2[:, m], w2[:, k, m], h1[:, k],
                                         start=(k == 0), stop=(k == 1))
                h2 = work.tile([128, 2, CN], F32R, tag=f"h2_{c}", name=f"h2_{c}")
                for m in range(2):
                    nc.scalar.activation(h2[:, m], ps2[:, m], TANH,
                                         bias=bt["b2"][:, m:m + 1])

                # ---- layer 3 (two single-bank tiles, alternated by stage
                # parity so deferred folds never bank-conflict with the next
                # stage's matmuls) ----
                if simple:
                    ps3b = psum.tile([DIM, 2, CN], F32, tag=f"ps3_{c}",
                                     name=f"ps3_{c}")
                    ps3 = ps3b
                else:
                    ps3b = psum.tile([DIM, 512], F32, tag=f"ps3{g % 2}_{c}",
                                     name=f"ps3{g % 2}_{c}")
                    ps3 = ps3b[:, 0:CN].rearrange("p (o n) -> p o n", o=1)
                for k in range(2):
                    nc.tensor.matmul(ps3[:, 0], w3[:, k], h2[:, k],
                                     start=(k == 0), stop=(k == 1))

                # ---- Runge-Kutta folds (DVE) ----
                # Critical now: z-stt and its immediate acc dependency.
                # Slack-tolerant folds (later accs, accy, y/dma/setups) are
                # deferred one stage so the scheduler priorities keep the
                # act-feeding path hot.
                deferred = []
                if s < 6:
                    nc.vector.scalar_tensor_tensor(
                        zaug[c][0:DIM, :], ps3[:, 0], sA(s - 1, t),
                        acc[c][s + 1][:], MULT, ADD)
                    if s + 2 <= 6:
                        q = 5 + PAIRQ[(s + 2, s)]
                        nc.vector.scalar_tensor_tensor(
                            acc[c][s + 2][:], ps3[:, 0], sA(q, t),
                            acc[c][s + 2][:], MULT, ADD)

                    def fold_rest(c=c, s=s, t=t, ps3=ps3):
                        for i in range(s + 3, 7):
                            q = 5 + PAIRQ[(i, s)]
                            nc.vector.scalar_tensor_tensor(
                                acc[c][i][:], ps3[:, 0], sA(q, t),
                                acc[c][i][:], MULT, ADD)
                        nc.vector.scalar_tensor_tensor(
                            accy[c][:], ps3[:, 0], sA(15 + s - 1, t),
                            accy[c][:], MULT, ADD)
                    if s == 5 or simple:
                        fold_rest()   # accy feeds stage 6 immediately
                    else:
                        deferred.append(fold_rest)
                else:
                    nc.vector.scalar_tensor_tensor(
                        zaug[c][0:DIM, :], ps3[:, 0], sA(20, t),
                        accy[c][:], MULT, ADD)

                    def tail(c=c, t=t, ps3=ps3):
                        yn = state.tile([DIM, CN], F32, tag=f"y_{c}", name=f"y_{c}")
                        nc.vector.scalar_tensor_tensor(
                            yn[:], ps3[:, 0], sA(20, t), accy[c][:], MULT, ADD)
                        nc.sync.dma_start(out_d[t][:, c * CN:(c + 1) * CN], yn[:])
                        if t + 1 < nsteps:
                            emit_setup(c, yn[:], t + 1)
                    if simple:
                        tail()
                    else:
                        deferred.append(tail)
                return deferred

            # chains run skewed by `skew` stages so each chain's MLP work
            # covers the other's inter-stage funnel; low-slack ops emit one
            # stage late to sit lower in the scheduler's priority heaps.
            nstages = 6 * nsteps
            pending = {c: [] for c in range(nch)}
            for g in range(nstages + skew * (nch - 1)):
                for c in range(nch):
                    gc = g - skew * c
                    if 0 <= gc < nstages:
                        prev = pending[c]
                        pending[c] = emit_stage(c, gc)
                        for fn in prev:
                            fn()
            for c in range(nch):
                for fn in pending[c]:
                    fn()

    nc.compile()
    return nc


def _get_nc(nsteps=NSTEP, **variant):
    key = (nsteps, tuple(sorted(variant.items())))
    if key not in _cache:
        _cache[key] = _build(nsteps, **variant)
    return _cache[key]


def _prepare_in_maps(ts, y0, W0, b0, W1, b1, W2, b2, W3, b3, nsteps=NSTEP):
    ts = np.asarray(ts, np.float32)
    hs = (ts[1:nsteps + 1] - ts[:nsteps]).astype(np.float64)          # [nsteps]
    b3v = np.asarray(b3, np.float64)
    # hA: [64, 21*nsteps]; q = 0..4: z-direct h*A_{i,i-1} (i=2..6);
    # q = 5..14: pre-acc h*A_ij per PAIRS; q = 15..19: h*B_j (j=1..5); q=20: h*B6
    AD = ACOEF
    cols = []
    for i in range(2, 7):
        cols.append(hs * AD[i][i - 1])
    for (i, j) in PAIRS:
        cols.append(hs * AD[i][j])
    for j in range(1, 6):
        cols.append(hs * BCOEF[j])
    cols.append(hs * B6)
    hA = np.concatenate([np.broadcast_to(c[None, :], (DIM, nsteps)) for c in cols],
                        axis=1).astype(np.float32)
    # hc: stage prefolds c_i*h*b3[d] (i=2..6) then (sum B)*h*b3[d]
    ccols = []
    for i in range(2, 7):
        ci = sum(AD[i].values())
        ccols.append(np.outer(b3v, hs * ci))
    ccols.append(np.outer(b3v, hs * sum(BCOEF.values())))
    hc = np.concatenate(ccols, axis=1).astype(np.float32)
    w0a = np.concatenate([np.asarray(W0, np.float32),
                          np.asarray(b0, np.float32)[None, :],
                          np.zeros((128 - DIM - 1, WIDTH), np.float32)], axis=0)
    common = {
        "hA": np.ascontiguousarray(hA), "hc": np.ascontiguousarray(hc),
        "W0A": np.ascontiguousarray(w0a),
        "W1": np.ascontiguousarray(W1, np.float32),
        "W2": np.ascontiguousarray(W2, np.float32),
        "W3": np.ascontiguousarray(W3, np.float32),
        "b1": np.ascontiguousarray(b1, np.float32),
        "b2": np.ascontiguousarray(b2, np.float32),
    }
    in_maps = []
    for i in range(NCORES):
        shard = np.asarray(y0[i * SHARD:(i + 1) * SHARD], np.float32)
        in_maps.append({"y0t": np.ascontiguousarray(shard.T), **common})
    return in_maps


def _run(inputs, nsteps=NSTEP, trace=False, **variant):
    nc = _get_nc(nsteps, **variant)
    in_maps = _prepare_in_maps(**inputs, nsteps=nsteps)
    res = run_bass_kernel_spmd(nc, in_maps, core_ids=list(range(NCORES)), trace=trace)
    y0 = np.asarray(inputs["y0"], np.float32)
    out = np.empty((nsteps + 1, BATCH, DIM), np.float32)
    out[0] = y0
    for i in range(NCORES):
        out[1:, i * SHARD:(i + 1) * SHARD, :] = res.results[i]["ysT"].transpose(0, 2, 1)
    return out, res


def kernel(**inputs) -> np.ndarray:
    out, _ = _run(inputs)
    return out


def _bench(inputs, iters=10, nsteps=NSTEP, **variant):
    """Time repeated device executes with a persistent jit + resident inputs."""
    import jax
    from jax.sharding import Mesh, PartitionSpec
    from jax.experimental.shard_map import shard_map
    from concourse import bass2jax
    from concourse import mybir as _mybir
    import time

    nc = _get_nc(nsteps, **variant)
    in_maps = _prepare_in_maps(**inputs, nsteps=nsteps)
    bass2jax.install_neuronx_cc_hook()

    partition_name = nc.partition_id_tensor.name if nc.partition_id_tensor else None
    in_names, out_names, out_avals = [], [], []
    for alloc in nc.m.functions[0].allocations:
        if not isinstance(alloc, _mybir.MemoryLocationSet):
            continue
        name = alloc.memorylocations[0].name
        if alloc.kind == "ExternalInput":
            if name != partition_name:
                in_names.append(name)
        elif alloc.kind == "ExternalOutput":
            out_names.append(name)
            out_avals.append(
                jax.core.ShapedArray(tuple(alloc.tensor_shape), _mybir.dt.np(alloc.dtype))
            )
    n_params = len(in_names)
    all_names = in_names + out_names
    if partition_name is not None:
        all_names = all_names + [partition_name]

    def _body(*args):
        operands = list(args)
        if partition_name is not None:
            operands.append(bass2jax.partition_id_tensor())
        return tuple(
            bass2jax._bass_exec_p.bind(
                *operands,
                out_avals=tuple(out_avals),
                in_names=tuple(all_names),
                out_names=tuple(out_names),
                lowering_input_output_aliases=(),
                sim_require_finite=True,
                sim_require_nnan=True,
                nc=nc,
            )
        )

    devices = jax.devices()[:NCORES]
    mesh = Mesh(np.asarray(devices), ("core",))
    n_outs = len(out_names)
    sharded = jax.jit(
        shard_map(
            _body,
            mesh=mesh,
            in_specs=(PartitionSpec("core"),) * (n_params + n_outs),
            out_specs=(PartitionSpec("core"),) * n_outs,
            check_rep=False,
        ),
        keep_unused=True,
    )
    concat_in = [
        jax.device_put(
            np.concatenate([np.asarray(in_maps[c][nm]) for c in range(NCORES)], axis=0)
        )
        for nm in in_names
    ]
    concat_zeros = [
        jax.device_put(np.zeros((NCORES * a.shape[0], *a.shape[1:]), a.dtype))
        for a in out_avals
    ]
    r = sharded(*concat_in, *concat_zeros)
    jax.block_until_ready(r)

    def run_n(n):
        t0 = time.perf_counter()
        rs = None
        for _ in range(n):
            rs = sharded(*concat_in, *concat_zeros)
        jax.block_until_ready(rs)
        return time.perf_counter() - t0

    run_n(3)  # pipeline warm
    slopes = []
    for _ in range(max(1, iters // 3)):
        t_small = run_n(5)
        t_big = run_n(25)
        slopes.append((t_big - t_small) / 20.0)
    return min(slopes), slopes
